# revision 6
# baseline (speedup 1.0000x reference)
"""Trainium2 Bass kernel for nn_AllLoss (6 chamfer distances + orthogonality
regularization) on 8 NeuronCores.

Strategy
--------
Data-parallel over batch B=8: core b computes batch b's chamfer terms; host
sums the 8 partial scalars (the all-reduce of the sharding hint) and adds the
tiny regularization term.

Math restructure: every chamfer direction becomes a rowmin-only KNN pass.
 * reflection distance matrices are symmetric (reflections are isometric
   involutions), so colmin == rowmin -> weight 2.
 * rotation colmin == rowmin of the inverse-rotated query cloud.
=> 9 query clouds (3 reflections, 3 rotations, 3 inverse rotations), each
needing rowmin_i = min_j D[i,j] over the same target cloud P, where
  D[i,j] = |T_i|^2 + |P_j|^2 - 2 T_i.P_j
         = TF[:,i] . PF[:,j],   TF = [-2T, aa, 1], PF = [P, 1, bb]  (K=5).

Retrieval structure: the host kd-sorts the points; for every (cloud, query
leaf of 32) it selects the S=16 target leaves (8 points each) minimizing the
true distance from the transformed query-leaf centroid to the leaf's points,
giving each query a W=128 candidate window.

Device graph (fully host-marshaled): the host builds, per (cloud, rgroup),
one [20, 4096] fp16 slab = [block-diagonal TF weight blocks | gathered PF
candidate features]. A meta-group mg = 4 query leaves shares one [20, 128]
block-diagonal weight block (leaf d's 5 TF rows nonzero only under its own
32 output columns) so one weight load + one matmul (K=20, M=128, N=W=128)
computes four independent per-leaf candidate windows. Meta-groups alternate
two PE row-bands (weight loads overlap in-flight matmuls); 16 windows fill
one 4-bank PSUM tile, and a single VectorEngine min-reduce consumes the tile
straight out of PSUM - the saturated DVE train is the kernel's floor. Host
zero-pads the weight blocks, so the device does no TF math, no scatter, no
memset: just 18 input DMAs, 288 matmuls, 18 reduces, 1 output DMA.
"""

import os
import sys

for _p in ("/opt/trn_rl_repo", "/root/.axon_site/_ro/trn_rl_repo"):
    if os.path.isdir(_p) and _p not in sys.path:
        sys.path.insert(0, _p)

import numpy as np

import concourse.bass as bass
import concourse.tile as tile
from concourse import bacc, mybir
from concourse.bass_utils import run_bass_kernel_spmd

EPS = 1e-8
WEIGHT = 25.0
B, N = 8, 4096
NC_ = 9          # query clouds
QL = 32          # queries per leaf
NQL = N // QL    # 128 query leaves
TL = 8           # points per target leaf
NTL = N // TL    # 512 target leaves
S = 16           # gathered target leaves per query leaf
W = S * TL       # candidates per query (128)
NMG = 32         # meta-groups (of 4 query leaves) per cloud
NT = 2           # psum tiles per cloud (16 windows each)
KDLEV = 9
F32 = mybir.dt.float32
F16 = mybir.dt.float16


# ----------------------------------------------------------------- host math
def _quat_R(quat):
    q = quat.astype(np.float64)
    q = q / (np.linalg.norm(q) + EPS)
    w, x, y, z = q
    K = np.array([[0, -z, y], [z, 0, -x], [-y, x, 0]], dtype=np.float64)
    return np.eye(3) + 2.0 * w * K + 2.0 * (K @ K)


def _transforms_for_batch(planes, quats):
    """9 (M, b) pairs: 3 reflections, 3 rotations, 3 inverse rotations.
    Row-vector convention: query = p @ M + b."""
    out = []
    for pl in planes:
        n = pl[:3].astype(np.float64)
        d = np.float64(pl[3])
        s = n @ n + EPS
        out.append((np.eye(3) - 2.0 * np.outer(n, n) / s, -(2.0 * d / s) * n))
    Rs = [_quat_R(q) for q in quats]
    for R in Rs:
        out.append((R.T, np.zeros(3)))
    for R in Rs:
        out.append((R, np.zeros(3)))
    return out


def kd_sort(P, levels=KDLEV):
    idx = np.arange(len(P))

    def rec(ids, depth):
        if depth == levels:
            return [ids]
        ax = depth % 3
        order = np.argsort(P[ids, ax], kind="stable")
        half = len(ids) // 2
        return rec(ids[order[:half]], depth + 1) + rec(ids[order[half:]], depth + 1)

    return np.concatenate(rec(idx, 0))


def _host_inputs_for_batch(points_b, planes, quats):
    """-> device input dict: one combined [NC_, 2, 20, 4096] fp16 slab of
    block-diagonal TF weights (cols 0:2048) + gathered PF features (2048:)."""
    perm = kd_sort(points_b.astype(np.float32))
    Ps = points_b.astype(np.float64)[perm]
    tfs = _transforms_for_batch(planes, quats)

    bb = (Ps * Ps).sum(-1)
    pf16 = np.empty((5, N), np.float16)
    pf16[0:3] = Ps.T
    pf16[3] = 1.0
    pf16[4] = bb

    qc0 = Ps.reshape(NQL, QL, 3).mean(axis=1)           # query leaf centroids
    ssq = bb  # |P_j|^2 reused for the centroid-to-point scores
    d4, a8 = np.arange(4), np.arange(TL)

    comb = np.zeros((NC_, 2, 20, 2 * N // 2), np.float16)  # [9, 2, 20, 4096]
    for k, (M, bvec) in enumerate(tfs):
        T = Ps @ M + bvec
        aa = (T * T).sum(-1)
        tf16 = np.empty((5, N), np.float16)
        tf16[0:3] = -2.0 * T.T
        tf16[3] = aa
        tf16[4] = 1.0

        # retrieval: true min distance from transformed query-leaf centroid
        # to each target leaf's points
        qc = qc0 @ M + bvec                              # [NQL, 3]
        d2 = (qc * qc).sum(-1)[:, None] + ssq[None, :] - 2.0 * (qc @ Ps.T)
        score = d2.reshape(NQL, NTL, TL).min(axis=2)     # [NQL, NTL]
        sel = np.argpartition(score, S - 1, axis=1)[:, :S]  # [NQL, S]
        cols = (sel[:, :, None] * TL + a8).reshape(NQL, W)  # [NQL, W]

        tf_v = tf16.reshape(5, 16, 2, 4, QL)             # (c, mp, r, d, q)
        for r in range(2):
            # weights: block-diagonal scatter, zeros prebuilt by np.zeros
            wt_v = comb[k, r, :, 0:2048].reshape(4, 5, 16, 4, QL)
            wt_v[d4, :, :, d4, :] = tf_v[:, :, r].transpose(2, 0, 1, 3)
            # slab: leaf (4*mg+d)'s gathered candidate features on rows 5d:5d+5
            ql_idx = ((np.arange(16) * 2 + r)[:, None] * 4 + d4).ravel()  # [64]
            gath = pf16[:, cols[ql_idx]]                 # [5, 64, W]
            comb[k, r, :, 2048:] = (
                gath.reshape(5, 16, 4, W).transpose(2, 0, 1, 3).reshape(20, 16 * W)
            )
    return {"comb": comb}


def _orth_loss_np(v1, v2, v3):
    def nrm(v):
        return v / (np.linalg.norm(v, axis=-1, keepdims=True) + EPS)

    M = np.stack([nrm(v1), nrm(v2), nrm(v3)], axis=1)
    G = np.einsum("bij,bkj->bik", M, M) - np.eye(3)
    return (G * G).sum(axis=(1, 2)).mean()


def _reg_loss_np(plane_x, plane_y, plane_z, rot_x, rot_y, rot_z):
    loss = _orth_loss_np(
        plane_x.astype(np.float64)[:, 0:3],
        plane_y.astype(np.float64)[:, 0:3],
        plane_z.astype(np.float64)[:, 0:3],
    )
    loss += _orth_loss_np(
        rot_x.astype(np.float64)[:, 1:4],
        rot_y.astype(np.float64)[:, 1:4],
        rot_z.astype(np.float64)[:, 1:4],
    )
    return loss


# ------------------------------------------------------------- device graph
# Tiles whose min-reduce goes straight from PSUM on the DVE (1 elem/cycle);
# the rest are staged: Scalar engine copies PSUM->SBUF fp16, then the DVE
# reduces from SBUF in its 4x perf mode. Balances the two engines at ~21us.
N_TILES = NC_ * NT
DIRECT = {round(x * N_TILES / 7) for x in range(7)}


def build_graph():
    nc = bacc.Bacc("TRN2", target_bir_lowering=False, debug=False)
    comb_d = nc.dram_tensor("comb", [NC_, 2, 20, 4096], F16, kind="ExternalInput").ap()
    out_d = nc.dram_tensor("out", [128, N_TILES * 16], F16, kind="ExternalOutput").ap()

    with tile.TileContext(nc) as tc:
        with (
            tc.tile_pool(name="const", bufs=1) as cpool,
            tc.tile_pool(name="slabs", bufs=3) as gpool,
            tc.tile_pool(name="stage", bufs=2) as spool,
            tc.tile_pool(name="psum", bufs=2, space="PSUM") as ppool,
        ):
            rowch = cpool.tile([128, N_TILES * 16], F16)

            dmai = 0
            for k in range(NC_):
                slab = gpool.tile([64, 4096], F16, tag="slab", name=f"slab{k}")
                for r in range(2):
                    eng = nc.sync if dmai % 2 == 0 else nc.gpsimd
                    dmai += 1
                    eng.dma_start(slab[32 * r : 32 * r + 20, :], comb_d[k, r])
                for t in range(NT):
                    gt = k * NT + t
                    pd = ppool.tile([128, 2048], F32, tag="d", name="pd")
                    for j in range(16):
                        mg = 16 * t + j
                        r = mg % 2
                        mp = mg // 2
                        # consecutive matmuls must cycle PSUM banks: 4
                        # back-to-back start/stop groups on one bank at
                        # different offsets error out on hardware
                        bk, h = j % 4, j // 4
                        nc.tensor.matmul(
                            pd[:, bk * 512 + h * 128 : bk * 512 + (h + 1) * 128],
                            slab[32 * r : 32 * r + 20, mp * 128 : (mp + 1) * 128],
                            slab[32 * r : 32 * r + 20, 2048 + mp * 128 : 2048 + (mp + 1) * 128],
                            start=True,
                            stop=True,
                            tile_position=(32 * r, 0),
                        )
                    if gt in DIRECT:
                        nc.vector.tensor_reduce(
                            rowch[:, gt * 16 : (gt + 1) * 16],
                            pd[:]
                            .rearrange("p (b x) -> p b x", x=512)
                            .rearrange("p b (h w) -> p b h w", w=128),
                            axis=mybir.AxisListType.X,
                            op=mybir.AluOpType.min,
                        )
                    else:
                        stg = spool.tile([128, 2048], F16, tag="s", name="stg")
                        nc.scalar.copy(stg[:], pd[:])
                        nc.vector.tensor_reduce(
                            rowch[:, gt * 16 : (gt + 1) * 16],
                            stg[:].rearrange("p (v w) -> p v w", w=128),
                            axis=mybir.AxisListType.X,
                            op=mybir.AluOpType.min,
                        )
            nc.sync.dma_start(out_d[:], rowch[:])

    nc.compile()
    return nc


_CACHE = {}


def _get_graph():
    if "nc" not in _CACHE:
        _CACHE["nc"] = build_graph()
    return _CACHE["nc"]


def unpack_rowmins(rm_flat):
    """[128, NC_*NT*16] -> [NC_, N] rowmins (relu'd) in sorted-query order.
    Window j of tile t lands at reduce position p = (j%4)*4 + j//4 (bank-major)
    and holds queries 128*(16t+j) + 32d + q on partition 32d+q."""
    rm = np.maximum(rm_flat.astype(np.float64).reshape(128, NC_, NT, 4, 4), 0.0)
    out = np.empty((NC_, N), np.float64)
    for k in range(NC_):
        for t in range(NT):
            for j in range(16):
                mg = 16 * t + j
                for d in range(4):
                    out[k, mg * 128 + d * 32 : mg * 128 + (d + 1) * 32] = rm[
                        32 * d : 32 * d + 32, k, t, j % 4, j // 4
                    ]
    return out


def combine_outputs(core_outs, inputs):
    total = 0.0
    for rm_flat in core_outs:
        s = unpack_rowmins(rm_flat.astype(np.float64)).sum(axis=1)  # [9]
        total += 2.0 * s[0:3].sum() + s[3:6].sum() + s[6:9].sum()
    loss = total / (B * N)
    loss += WEIGHT * _reg_loss_np(
        inputs["plane_x"],
        inputs["plane_y"],
        inputs["plane_z"],
        inputs["rot_x"],
        inputs["rot_y"],
        inputs["rot_z"],
    )
    return np.array([loss], dtype=np.float32)


def make_in_maps(inputs):
    in_maps = []
    for b in range(B):
        planes = [inputs["plane_x"][b], inputs["plane_y"][b], inputs["plane_z"][b]]
        quats = [inputs["rot_x"][b], inputs["rot_y"][b], inputs["rot_z"][b]]
        in_maps.append(_host_inputs_for_batch(inputs["points"][b], planes, quats))
    return in_maps


def kernel(**inputs):
    inputs = {k: np.asarray(v) for k, v in inputs.items()}
    nc = _get_graph()
    in_maps = make_in_maps(inputs)
    res = run_bass_kernel_spmd(nc, in_maps, core_ids=list(range(8)))
    core_outs = [res.results[i]["out"] for i in range(8)]
    return combine_outputs(core_outs, inputs)


if __name__ == "__main__":
    build_graph()
    print("graph built and compiled OK")


# revision 7
# speedup vs baseline: 1.4398x; 1.4398x over previous
"""Trainium2 Bass kernel for nn_AllLoss (6 chamfer distances + orthogonality
regularization) on 8 NeuronCores.

Strategy
--------
Data-parallel over batch B=8: core b computes batch b's chamfer terms; host
sums the 8 partial scalars (the all-reduce of the sharding hint) and adds the
tiny regularization term.

Math restructure: every chamfer direction becomes a rowmin-only KNN pass.
 * reflection distance matrices are symmetric (reflections are isometric
   involutions), so colmin == rowmin -> weight 2.
 * rotation colmin == rowmin of the inverse-rotated query cloud.
=> 9 query clouds (3 reflections, 3 rotations, 3 inverse rotations), each
needing rowmin_i = min_j D[i,j] over the same target cloud P, where
  D[i,j] = |T_i|^2 + |P_j|^2 - 2 T_i.P_j
         = TF[:,i] . PF[:,j],   TF = [-2T, aa, 1], PF = [P, 1, bb]  (K=5).

Retrieval structure: the host kd-sorts the points; for every (cloud, query
leaf of 16) it selects the S=16 target leaves (4 points each) minimizing the
true distance from the transformed query-leaf centroid to the leaf's points,
giving each query a W=64 candidate window.

Device graph (fully host-marshaled): the host builds, per (cloud, rgroup),
one [40, 3072] fp16 slab = [block-diagonal TF weight blocks | gathered PF
candidate features]. A meta-group mg = 8 query leaves shares one [40, 128]
block-diagonal weight block (leaf d's 5 TF rows nonzero only under its own
16 output columns) so one weight load + one matmul (K=40, M=128, N=W=64)
computes eight independent per-leaf candidate windows. Meta-groups alternate
two PE row-bands at rows 0/64 (weight loads overlap in-flight matmuls); one
cloud's 32 windows perfectly fill a 4-bank PSUM tile (8 windows of 64 per
512-col bank), and a single VectorEngine min-reduce consumes the whole cloud
straight out of PSUM - the saturated DVE train and the PE weight-load train
are co-critical at ~21us. Host zero-pads the weight blocks, so the device
does no TF math, no scatter, no memset: 18 input DMAs, 288 matmuls, 9
reduces, 1 output DMA.
"""

import os
import sys

for _p in ("/opt/trn_rl_repo", "/root/.axon_site/_ro/trn_rl_repo"):
    if os.path.isdir(_p) and _p not in sys.path:
        sys.path.insert(0, _p)

import numpy as np

import concourse.bass as bass
import concourse.tile as tile
from concourse import bacc, mybir
from concourse.bass_utils import run_bass_kernel_spmd

EPS = 1e-8
WEIGHT = 25.0
B, N = 8, 4096
NC_ = 9          # query clouds
QL = 16          # queries per leaf
NQL = N // QL    # 256 query leaves
TL = 4           # points per target leaf
NTL = N // TL    # 1024 target leaves
S = 16           # gathered target leaves per query leaf
W = S * TL       # candidates per query (64)
NMG = 32         # meta-groups (of 8 query leaves) per cloud
KDLEV = 10
F32 = mybir.dt.float32
F16 = mybir.dt.float16
WTC = NMG // 2 * 128     # weight columns per rgroup slab (2048)
PGC = NMG // 2 * W       # feature columns per rgroup slab (1024)


# ----------------------------------------------------------------- host math
def _quat_R(quat):
    q = quat.astype(np.float64)
    q = q / (np.linalg.norm(q) + EPS)
    w, x, y, z = q
    K = np.array([[0, -z, y], [z, 0, -x], [-y, x, 0]], dtype=np.float64)
    return np.eye(3) + 2.0 * w * K + 2.0 * (K @ K)


def _transforms_for_batch(planes, quats):
    """9 (M, b) pairs: 3 reflections, 3 rotations, 3 inverse rotations.
    Row-vector convention: query = p @ M + b."""
    out = []
    for pl in planes:
        n = pl[:3].astype(np.float64)
        d = np.float64(pl[3])
        s = n @ n + EPS
        out.append((np.eye(3) - 2.0 * np.outer(n, n) / s, -(2.0 * d / s) * n))
    Rs = [_quat_R(q) for q in quats]
    for R in Rs:
        out.append((R.T, np.zeros(3)))
    for R in Rs:
        out.append((R, np.zeros(3)))
    return out


def kd_sort(P, levels=KDLEV):
    idx = np.arange(len(P))

    def rec(ids, depth):
        if depth == levels:
            return [ids]
        ax = depth % 3
        order = np.argsort(P[ids, ax], kind="stable")
        half = len(ids) // 2
        return rec(ids[order[:half]], depth + 1) + rec(ids[order[half:]], depth + 1)

    return np.concatenate(rec(idx, 0))


def _host_inputs_for_batch(points_b, planes, quats):
    """-> device input dict: one combined [NC_, 2, 40, 3072] fp16 slab of
    block-diagonal TF weights (cols 0:2048) + gathered PF features (2048:)."""
    perm = kd_sort(points_b.astype(np.float32))
    Ps = points_b.astype(np.float64)[perm]
    tfs = _transforms_for_batch(planes, quats)

    bb = (Ps * Ps).sum(-1)
    pf16 = np.empty((5, N), np.float16)
    pf16[0:3] = Ps.T
    pf16[3] = 1.0
    pf16[4] = bb

    qc0 = Ps.reshape(NQL, QL, 3).mean(axis=1)           # query leaf centroids
    d8, aT = np.arange(8), np.arange(TL)

    comb = np.zeros((NC_, 2, 40, WTC + PGC), np.float16)
    for k, (M, bvec) in enumerate(tfs):
        T = Ps @ M + bvec
        aa = (T * T).sum(-1)
        tf16 = np.empty((5, N), np.float16)
        tf16[0:3] = -2.0 * T.T
        tf16[3] = aa
        tf16[4] = 1.0

        # retrieval: true min distance from transformed query-leaf centroid
        # to each target leaf's points
        qc = qc0 @ M + bvec                              # [NQL, 3]
        d2 = (qc * qc).sum(-1)[:, None] + bb[None, :] - 2.0 * (qc @ Ps.T)
        score = d2.reshape(NQL, NTL, TL).min(axis=2)     # [NQL, NTL]
        sel = np.argpartition(score, S - 1, axis=1)[:, :S]  # [NQL, S]
        cols = (sel[:, :, None] * TL + aT).reshape(NQL, W)  # [NQL, W]

        tf_v = tf16.reshape(5, 16, 2, 8, QL)             # (c, mp, r, d, q)
        for r in range(2):
            # weights: block-diagonal scatter, zeros prebuilt by np.zeros
            wt_v = comb[k, r, :, 0:WTC].reshape(8, 5, 16, 8, QL)
            wt_v[d8, :, :, d8, :] = tf_v[:, :, r].transpose(2, 0, 1, 3)
            # slab: leaf (8*mg+d)'s gathered candidate features on rows 5d:5d+5
            ql_idx = ((np.arange(16) * 2 + r)[:, None] * 8 + d8).ravel()  # [128]
            gath = pf16[:, cols[ql_idx]]                 # [5, 128, W]
            comb[k, r, :, WTC:] = (
                gath.reshape(5, 16, 8, W).transpose(2, 0, 1, 3).reshape(40, 16 * W)
            )
    return {"comb": comb}


def _orth_loss_np(v1, v2, v3):
    def nrm(v):
        return v / (np.linalg.norm(v, axis=-1, keepdims=True) + EPS)

    M = np.stack([nrm(v1), nrm(v2), nrm(v3)], axis=1)
    G = np.einsum("bij,bkj->bik", M, M) - np.eye(3)
    return (G * G).sum(axis=(1, 2)).mean()


def _reg_loss_np(plane_x, plane_y, plane_z, rot_x, rot_y, rot_z):
    loss = _orth_loss_np(
        plane_x.astype(np.float64)[:, 0:3],
        plane_y.astype(np.float64)[:, 0:3],
        plane_z.astype(np.float64)[:, 0:3],
    )
    loss += _orth_loss_np(
        rot_x.astype(np.float64)[:, 1:4],
        rot_y.astype(np.float64)[:, 1:4],
        rot_z.astype(np.float64)[:, 1:4],
    )
    return loss


# ------------------------------------------------------------- device graph
def build_graph():
    nc = bacc.Bacc("TRN2", target_bir_lowering=False, debug=False)
    comb_d = nc.dram_tensor(
        "comb", [NC_, 2, 40, WTC + PGC], F16, kind="ExternalInput"
    ).ap()
    out_d = nc.dram_tensor("out", [128, NC_ * NMG], F16, kind="ExternalOutput").ap()

    with tile.TileContext(nc) as tc:
        with (
            tc.tile_pool(name="const", bufs=1) as cpool,
            tc.tile_pool(name="slabs", bufs=3) as gpool,
            tc.tile_pool(name="psum", bufs=2, space="PSUM") as ppool,
        ):
            rowch = cpool.tile([128, NC_ * NMG], F16)

            dmai = 0
            for k in range(NC_):
                slab = gpool.tile([128, WTC + PGC], F16, tag="slab", name=f"slab{k}")
                for r in range(2):
                    eng = nc.sync if dmai % 2 == 0 else nc.scalar
                    dmai += 1
                    eng.dma_start(slab[64 * r : 64 * r + 40, :], comb_d[k, r])
                pd = ppool.tile([128, 2048], F32, tag="d", name="pd")
                for mg in range(NMG):
                    r = mg % 2
                    mp = mg // 2
                    # consecutive matmuls must cycle PSUM banks: back-to-back
                    # start/stop groups on one bank at different offsets
                    # error out on hardware
                    bk, h = mg % 4, mg // 4
                    nc.tensor.matmul(
                        pd[:, bk * 512 + h * W : bk * 512 + (h + 1) * W],
                        slab[64 * r : 64 * r + 40, mp * 128 : (mp + 1) * 128],
                        slab[64 * r : 64 * r + 40, WTC + mp * W : WTC + (mp + 1) * W],
                        start=True,
                        stop=True,
                        tile_position=(64 * r, 0),
                    )
                nc.vector.tensor_reduce(
                    rowch[:, k * NMG : (k + 1) * NMG],
                    pd[:]
                    .rearrange("p (b x) -> p b x", x=512)
                    .rearrange("p b (h w) -> p b h w", w=W),
                    axis=mybir.AxisListType.X,
                    op=mybir.AluOpType.min,
                )
            nc.sync.dma_start(out_d[:], rowch[:])

    nc.compile()
    return nc


_CACHE = {}


def _get_graph():
    if "nc" not in _CACHE:
        _CACHE["nc"] = build_graph()
    return _CACHE["nc"]


def unpack_rowmins(rm_flat):
    """[128, NC_*NMG] -> [NC_, N] rowmins (relu'd) in sorted-query order.
    Window mg of cloud k lands at reduce position p = (mg%4)*8 + mg//4
    (bank-major) and holds query 128*mg + 16d + q on partition 16d+q."""
    rm = np.maximum(rm_flat.astype(np.float64).reshape(128, NC_, NMG), 0.0)
    out = np.empty((NC_, N), np.float64)
    for k in range(NC_):
        for mg in range(NMG):
            p = (mg % 4) * 8 + mg // 4
            out[k, mg * 128 : (mg + 1) * 128] = rm[:, k, p]
    return out


def combine_outputs(core_outs, inputs):
    total = 0.0
    for rm_flat in core_outs:
        s = unpack_rowmins(rm_flat.astype(np.float64)).sum(axis=1)  # [9]
        total += 2.0 * s[0:3].sum() + s[3:6].sum() + s[6:9].sum()
    loss = total / (B * N)
    loss += WEIGHT * _reg_loss_np(
        inputs["plane_x"],
        inputs["plane_y"],
        inputs["plane_z"],
        inputs["rot_x"],
        inputs["rot_y"],
        inputs["rot_z"],
    )
    return np.array([loss], dtype=np.float32)


def make_in_maps(inputs):
    in_maps = []
    for b in range(B):
        planes = [inputs["plane_x"][b], inputs["plane_y"][b], inputs["plane_z"][b]]
        quats = [inputs["rot_x"][b], inputs["rot_y"][b], inputs["rot_z"][b]]
        in_maps.append(_host_inputs_for_batch(inputs["points"][b], planes, quats))
    return in_maps


def kernel(**inputs):
    inputs = {k: np.asarray(v) for k, v in inputs.items()}
    nc = _get_graph()
    in_maps = make_in_maps(inputs)
    res = run_bass_kernel_spmd(nc, in_maps, core_ids=list(range(8)))
    core_outs = [res.results[i]["out"] for i in range(8)]
    return combine_outputs(core_outs, inputs)


if __name__ == "__main__":
    build_graph()
    print("graph built and compiled OK")
